# revision 26
# baseline (speedup 1.0000x reference)
"""HDC Level Encoder kernel for 8 Trainium2 NeuronCores.

Strategy (D=100000 hypervector dim sharded 8 ways, padded to 12800/core):
  - level-table lookups as one-hot matmuls on PE. Tables and one-hots ship as
    fp8e4 (+-1 and 0/1 are exact); x/y/z lookups accumulate the bundle sum
    directly in PSUM (f32, exact).
  - bind with time hv on DVE, multibind product over the N=128 window via PE
    transpose to d-on-partitions layout + pairwise DVE fold tree (f32, same
    rounding as the reference's f32 product chain).
  - Sinusoid einsum as f32 PE matmuls with the WEIGHT CHUNK STATIONARY
    (lhsT = W[rows, 128 d-cols]) and the block-diagonal feature matrix moving
    (rhs [rows, 24]): the f32 4-cycles/row penalty applies to the 24-wide
    moving operand instead of a 512-wide one, and the result lands directly
    in d-on-partitions layout (no transpose). Accumulation order over the
    contraction rows/chunks is identical to the reference einsum.
  - cos(p+b)*sin(p) via ScalarE Sin with explicit range reduction in cycle
    units: m = u - rint(u); Sin(2*pi*m) = sin(2*pi*u); bsh carries b/(2*pi)
    + 0.25 so the cos becomes the same shifted sin.
  - combine + hard_quantize on DVE, transpose back, DMA out.

Perf shape: per-DMA overhead (HWDGE + issuing-sequencer hold) is ~0.7us, so
tables and weights are HOST-PACKED into one contiguous [partitions, bytes]
block per 512-wide chunk -> exactly 2 DMAs per chunk. DVE dispatch is
~0.4us/op, so the fold tree and the trig chain are batched across groups of
5 chunks. Host does only O(N*levels + K*D) layout prep: index math
(bit-identical to the reference's f32 ops), one-hot construction, weight
restack/padding, and sharding.
"""

import sys

for _p in ("/opt/trn_rl_repo",):
    if _p not in sys.path:
        sys.path.insert(0, _p)

import numpy as np

import concourse.bacc as bacc
import concourse.mybir as mybir
import concourse.tile as tile
from concourse import bass_utils, masks

F32 = mybir.dt.float32
I32 = mybir.dt.int32
BF = mybir.dt.bfloat16
FP8 = mybir.dt.float8e4
FP8NP = mybir.dt.np(FP8)
AF = mybir.AluOpType

D = 100000          # true hypervector dim
NCORES = 8
DC = 12800          # per-core padded dim
DP = DC * NCORES    # 102400
N = 128             # window length
LEVELS = 100
TIMESTAMPS = 128
CH = 25             # chunks of 512 per core
CW = 512            # chunk width
GRP = 5             # chunks per fold/trig batch group
NSUB = CH * 4       # 100 sub-chunks of 128
NK = 24             # sinusoid kernels (6 big + 18 small)
KROWS = 600         # stacked contraction dim (6*91 + 18*3)
KB = 5              # contraction row blocks
KR = KROWS // KB    # 120 rows per block

_TWO_PI = np.float32(2.0 * np.pi)
_INV_2PI = np.float32(1.0 / (2.0 * np.pi))

_nc_cache = {}

# (row0, nrows) blocks of the stacked table tensor: x, y, z, t
TBLOCKS = [(0, LEVELS), (LEVELS, LEVELS), (2 * LEVELS, LEVELS), (3 * LEVELS, TIMESTAMPS)]


def _build_nc():
    nc = bacc.Bacc("TRN2", target_bir_lowering=False, debug=False)

    # tabs4: per chunk c, cols [c*2048, (c+1)*2048) hold the 4 table blocks
    # (x,y,z,t), each [rows<=128 on partitions, 512], zero-padded to 128 rows.
    tabs4 = nc.dram_tensor("tabs4", [128, CH * 4 * CW], FP8, kind="ExternalInput")
    # oh4: 4 one-hot lhsT blocks [rows, 128] at col b*128
    oh4 = nc.dram_tensor("oh4", [128, 4 * N], FP8, kind="ExternalInput")
    # wpack: per chunk c, cols [c*2560, (c+1)*2560) hold 5 K-blocks of
    # [120 rows on partitions, 512 d]
    wpack = nc.dram_tensor("wpack", [KR, CH * KB * CW], F32, kind="ExternalInput")
    # fbd2: 5 K-blocks of the block-diagonal feature matrix, [120, 24] each
    fbd2 = nc.dram_tensor("fbd2", [KR, KB * NK], F32, kind="ExternalInput")
    bsh = nc.dram_tensor("bsh", [N, NSUB * NK], F32, kind="ExternalInput")
    out = nc.dram_tensor("out", [NSUB, N], F32, kind="ExternalOutput")

    with tile.TileContext(nc) as tc:
        with (
            tc.tile_pool(name="const", bufs=1) as constp,
            tc.tile_pool(name="grand", bufs=1) as grandp,
        ):
            ident_bf = constp.tile([128, 128], BF)
            masks.make_identity(nc, ident_bf[:])
            ident_f32 = constp.tile([128, 128], F32)
            masks.make_identity(nc, ident_f32[:])

            oh_sb = constp.tile([128, 4 * N], FP8, tag="oh4")
            nc.sync.dma_start(oh_sb[:], oh4.ap())
            fbd_sb = constp.tile([KR, KB * NK], F32, tag="fbd2")
            nc.sync.dma_start(fbd_sb[:], fbd2.ap())

            outq_all = grandp.tile([128, NSUB], F32, tag="outq_all")

            with (
                tc.tile_pool(name="tabs", bufs=4) as tabp,
                tc.tile_pool(name="wts", bufs=4) as wp,
                tc.tile_pool(name="binds", bufs=3) as bindp,
                tc.tile_pool(name="bt5s", bufs=2) as btp,
                tc.tile_pool(name="folds", bufs=2) as foldp,
                tc.tile_pool(name="trig", bufs=2) as trp,
                tc.tile_pool(name="comb", bufs=2) as cp,
                tc.tile_pool(name="psA", bufs=2, space="PSUM") as psa,
                tc.tile_pool(name="psB", bufs=2, space="PSUM") as psb,
            ):
                for grp in range(CH // GRP):
                    ppt_g = psb.tile([128, GRP * 4 * NK], F32, tag="ppt")
                    bt5 = btp.tile([128, GRP * CW], BF, tag="bt5")
                    GW = GRP * 4 * NK
                    bsh_t = trp.tile([N, GW], F32, tag="bsh_t")
                    nc.sync.dma_start(
                        bsh_t[:], bsh.ap()[:, grp * GW : (grp + 1) * GW]
                    )
                    for g in range(GRP):
                        c = grp * GRP + g
                        tab_t = tabp.tile([128, 4 * CW], FP8, tag="tab")
                        nc.sync.dma_start(
                            tab_t[:], tabs4.ap()[:, c * 4 * CW : (c + 1) * 4 * CW]
                        )
                        w_t = wp.tile([KR, KB * CW], F32, tag="w")
                        nc.scalar.dma_start(
                            w_t[:], wpack.ap()[:, c * KB * CW : (c + 1) * KB * CW]
                        )

                        # ---- phase A: lookups, bind ----------------------
                        ps3 = psa.tile([128, CW], F32, tag="ps3")
                        for b in range(3):
                            nc.tensor.matmul(
                                ps3[:],
                                oh_sb[0:LEVELS, b * N : (b + 1) * N],
                                tab_t[0:LEVELS, b * CW : (b + 1) * CW],
                                start=(b == 0),
                                stop=(b == 2),
                            )
                        ptl = psa.tile([128, CW], F32, tag="ptl")
                        nc.tensor.matmul(
                            ptl[:],
                            oh_sb[0:TIMESTAMPS, 3 * N : 4 * N],
                            tab_t[0:TIMESTAMPS, 3 * CW : 4 * CW],
                            start=True,
                            stop=True,
                        )

                        tl_sb = bindp.tile([128, CW], BF, tag="tl_sb")
                        nc.scalar.copy(tl_sb[:], ptl[:])
                        bind_sb = bindp.tile([128, CW], BF, tag="bind_sb")
                        nc.vector.tensor_mul(bind_sb[:], ps3[:], tl_sb[:])

                        pbt = psa.tile([128, CW], BF, tag="pbt")
                        for s in range(4):
                            ss = slice(s * 128, (s + 1) * 128)
                            nc.tensor.matmul(
                                pbt[:, ss], bind_sb[:, ss], ident_bf[:],
                                is_transpose=True,
                            )
                        nc.scalar.copy(bt5[:, g * CW : (g + 1) * CW], pbt[:])

                        # ---- phase B: einsum, W chunk stationary ---------
                        for s in range(4):
                            od = slice(g * 4 * NK + s * NK, g * 4 * NK + (s + 1) * NK)
                            for i in range(KB):
                                nc.tensor.matmul(
                                    ppt_g[:, od],
                                    w_t[:, i * CW + s * 128 : i * CW + (s + 1) * 128],
                                    fbd_sb[:, i * NK : (i + 1) * NK],
                                    start=(i == 0),
                                    stop=(i == KB - 1),
                                )

                    # ---- group tail: fold tree over the window dim -------
                    src = bt5[:].rearrange("p (s n) -> p s n", s=4 * GRP)
                    hv_t = foldp.tile([128, 4 * GRP], F32, tag="hv")
                    w = 64
                    while w >= 1:
                        if w == 1:
                            dst_ap = hv_t[:].rearrange("p (s n) -> p s n", n=1)
                        else:
                            t_new = foldp.tile([128, 4 * GRP * w], F32, tag=f"fold{w}")
                            dst_ap = t_new[:].rearrange("p (s n) -> p s n", s=4 * GRP)
                        nc.vector.tensor_mul(
                            dst_ap, src[:, :, 0:w], src[:, :, w : 2 * w]
                        )
                        if w > 1:
                            src = dst_ap
                        w //= 2

                    # ---- group tail: trig ---------------------------------
                    # range reduction in cycle units: r = u - rint(u) in
                    # [-0.5, 0.5] (DVE f32->int32 copy rounds half-to-even,
                    # and the subtraction is exact), then Sin(2*pi*r) =
                    # sin(2*pi*u) on ScalarE's [-pi, pi] domain; the 2*pi
                    # multiply is fused into the activation's scale (same
                    # f32 product the reference rounds).
                    u = trp.tile([128, GW], F32, tag="u")
                    nc.vector.tensor_scalar_mul(u[:], ppt_g[:], float(_INV_2PI))
                    i1 = trp.tile([128, GW], I32, tag="i1")
                    nc.vector.tensor_copy(i1[:], u[:])
                    m1 = trp.tile([128, GW], F32, tag="m1")
                    nc.vector.tensor_sub(m1[:], u[:], i1[:])
                    s1 = trp.tile([128, GW], F32, tag="s1")
                    nc.scalar.activation(
                        s1[:], m1[:], mybir.ActivationFunctionType.Sin,
                        scale=float(_TWO_PI),
                    )
                    u2 = trp.tile([128, GW], F32, tag="u2")
                    nc.vector.tensor_add(u2[:], u[:], bsh_t[:])
                    i2 = trp.tile([128, GW], I32, tag="i2")
                    nc.vector.tensor_copy(i2[:], u2[:])
                    m2 = trp.tile([128, GW], F32, tag="m2")
                    nc.vector.tensor_sub(m2[:], u2[:], i2[:])
                    s2 = trp.tile([128, GW], F32, tag="s2")
                    nc.scalar.activation(
                        s2[:], m2[:], mybir.ActivationFunctionType.Sin,
                        scale=float(_TWO_PI),
                    )
                    fg_t = trp.tile([128, GW], F32, tag="fg")
                    nc.vector.tensor_mul(fg_t[:], s2[:], s1[:])

                    # ---- group tail: combine + hard quantize --------------
                    # t2's factor chains run on the (otherwise idle) Pool
                    # engine; t1's on DVE. All ops are scalar IEEE f32
                    # mul/add, same order as the reference formula.
                    f3 = fg_t[:].rearrange("p (s k) -> p s k", k=NK)

                    def f(k):
                        return f3[:, :, k : k + 1]

                    def tmp(tag):
                        return cp.tile([128, 4 * GRP], F32, tag=tag, name=tag)

                    hvv = hv_t[:].rearrange("p (s k) -> p s k", k=1)
                    a1 = tmp("a1")
                    a1v = a1[:].rearrange("p (s k) -> p s k", k=1)
                    nc.vector.tensor_add(a1v, f(6), f(21))
                    nc.vector.tensor_add(a1v, a1v, f(23))
                    q1 = tmp("q1")
                    q1v = q1[:].rearrange("p (s k) -> p s k", k=1)
                    nc.vector.tensor_mul(q1v, hvv, a1v)
                    a2 = tmp("a2")
                    a2v = a2[:].rearrange("p (s k) -> p s k", k=1)
                    nc.vector.tensor_add(a2v, f(9), f(10))
                    nc.vector.tensor_mul(q1v, q1v, a2v)
                    for k in (11, 12, 17, 18):
                        nc.vector.tensor_mul(q1v, q1v, f(k))

                    a3 = tmp("a3")
                    a3v = a3[:].rearrange("p (s k) -> p s k", k=1)
                    nc.gpsimd.tensor_add(a3v, f(6), f(10))
                    nc.gpsimd.tensor_add(a3v, a3v, f(11))
                    nc.gpsimd.tensor_add(a3v, a3v, f(12))
                    p2 = tmp("p2")
                    p2v = p2[:].rearrange("p (s k) -> p s k", k=1)
                    nc.gpsimd.tensor_mul(p2v, f(0), f(1))
                    for k in (2, 3, 4, 5):
                        nc.gpsimd.tensor_mul(p2v, p2v, f(k))
                    q2 = tmp("q2")
                    q2v = q2[:].rearrange("p (s k) -> p s k", k=1)
                    nc.gpsimd.tensor_mul(q2v, hvv, a3v)
                    nc.gpsimd.tensor_mul(q2v, q2v, p2v)

                    comb = tmp("comb")
                    nc.vector.tensor_add(comb[:], q1[:], q2[:])
                    oq = slice(grp * 4 * GRP, (grp + 1) * 4 * GRP)
                    nc.vector.tensor_scalar(
                        comb[:], comb[:], 0.0, 2.0, AF.is_gt, AF.mult
                    )
                    nc.vector.tensor_scalar(
                        outq_all[:, oq], comb[:], -1.0, None, AF.add
                    )

            # ---------------- transpose back + output -------------------------
            with (
                tc.tile_pool(name="outp", bufs=1) as cpo,
                tc.tile_pool(name="psC", bufs=1, space="PSUM") as psc,
            ):
                pso = psc.tile([NSUB, 128], F32, tag="pso")
                nc.tensor.matmul(
                    pso[:], outq_all[:], ident_f32[:], is_transpose=True
                )
                out_sb = cpo.tile([NSUB, 128], F32, tag="out_sb")
                nc.scalar.copy(out_sb[:], pso[:])
                nc.sync.dma_start(out.ap(), out_sb[:])

    nc.compile()
    return nc


def _get_nc():
    if "nc" not in _nc_cache:
        _nc_cache["nc"] = _build_nc()
    return _nc_cache["nc"]


def _value_to_index(x, low, high, num):
    """Bit-identical (f32 elementwise IEEE ops) to the reference's jnp math."""
    x = x.astype(np.float32)
    xc = np.clip(x, np.float32(low), np.float32(high))
    t = (xc - np.float32(low)) / np.float32(high - low) * np.float32(num - 1)
    idx = np.round(t)  # round-half-even, same as jnp.round
    return np.clip(idx, 0, num - 1).astype(np.int32)


def prepare_in_maps(
    input,
    feat,
    level_x,
    level_y,
    level_z,
    level_t,
    W_big,
    b_big,
    W_small,
    b_small,
):
    ix = _value_to_index(input[:, 1], -5.0, 5.0, LEVELS)
    iy = _value_to_index(input[:, 2], -5.0, 5.0, LEVELS)
    iz = _value_to_index(input[:, 3], -5.0, 5.0, LEVELS)
    it = _value_to_index(input[:, 0], 0.0, float(TIMESTAMPS), TIMESTAMPS)

    # one-hot lhsT blocks [rows, 128] packed at col b*128 (0/1 exact in fp8)
    oh4 = np.zeros((128, 4 * N), dtype=FP8NP)
    for bi, idx in enumerate((ix, iy, iz, it)):
        oh4[idx, bi * N + np.arange(N)] = 1

    featb = feat[:546].reshape(6, 91).astype(np.float32)
    feats = feat[546:600].reshape(18, 3).astype(np.float32)
    fbd = np.zeros((KROWS, NK), dtype=np.float32)
    for k in range(6):
        fbd[k * 91 : (k + 1) * 91, k] = featb[k]
    for k in range(18):
        fbd[546 + k * 3 : 546 + (k + 1) * 3, 6 + k] = feats[k]
    # 5 row-blocks of 120, side by side: [120, 5*24]
    fbd2 = np.ascontiguousarray(
        fbd.reshape(KB, KR, NK).transpose(1, 0, 2).reshape(KR, KB * NK)
    )

    def padD(a):
        w = [(0, 0)] * a.ndim
        w[-1] = (0, DP - D)
        return np.pad(a, w)

    tables = [
        padD(t).astype(FP8NP) for t in (level_x, level_y, level_z, level_t)
    ]

    # W stack [600, DP] f32: rows = (kernel-major, in-feature) of W_big/W_small
    wb = np.ascontiguousarray(W_big.transpose(0, 2, 1)).reshape(546, D)
    ws = np.ascontiguousarray(W_small.transpose(0, 2, 1)).reshape(54, D)
    wstk = padD(np.concatenate([wb, ws], axis=0)).astype(np.float32)

    # b shift in cycles (+0.25 for the cos->sin shift), d-on-partitions layout
    ball = np.concatenate([b_big, b_small], axis=0).astype(np.float64)
    bsh_full = padD((ball / (2.0 * np.pi) + 0.25).astype(np.float32))  # [24, DP]

    in_maps = []
    for ci in range(NCORES):
        ds = slice(ci * DC, (ci + 1) * DC)
        # pack the 4 table blocks: [128, CH, 4, 512]
        tabs4 = np.zeros((128, CH, 4, CW), dtype=FP8NP)
        for bi, tab in enumerate(tables):
            rn = TBLOCKS[bi][1]
            tabs4[0:rn, :, bi, :] = tab[:, ds].reshape(rn, CH, CW)
        # pack the 5 W row-blocks: [120, CH, 5, 512]
        wp_ = np.empty((KR, CH, KB, CW), dtype=np.float32)
        wc = wstk[:, ds]
        for i in range(KB):
            wp_[:, :, i, :] = wc[i * KR : (i + 1) * KR].reshape(KR, CH, CW)
        bs = (
            bsh_full[:, ds]
            .reshape(NK, NSUB, 128)
            .transpose(2, 1, 0)
            .reshape(128, NSUB * NK)
        )
        in_maps.append(
            {
                "tabs4": np.ascontiguousarray(tabs4.reshape(128, CH * 4 * CW)),
                "oh4": oh4,
                "wpack": np.ascontiguousarray(wp_.reshape(KR, CH * KB * CW)),
                "fbd2": fbd2,
                "bsh": np.ascontiguousarray(bs),
            }
        )
    return in_maps


def kernel(**inputs):
    nc = _get_nc()
    in_maps = prepare_in_maps(**inputs)
    _nc_cache["last_in_maps"] = in_maps
    res = bass_utils.run_bass_kernel_spmd(nc, in_maps, core_ids=list(range(NCORES)))
    shards = [res.results[ci]["out"].reshape(-1) for ci in range(NCORES)]
    return np.concatenate(shards)[:D].astype(np.float32)


# revision 36
# speedup vs baseline: 2.8823x; 2.8823x over previous
"""HDC Level Encoder kernel for 8 Trainium2 NeuronCores.

Strategy (D=100000 hypervector dim sharded 8 ways, padded to 12800/core):
  - level-table lookups as one-hot matmuls on PE. Tables and one-hots ship as
    fp8e4 (+-1 and 0/1 are exact); x/y/z lookups accumulate the bundle sum
    directly in PSUM (f32, exact).
  - bind with time hv on DVE, multibind product over the N=128 window via PE
    transpose to d-on-partitions layout + pairwise DVE fold tree (f32, same
    rounding as the reference's f32 product chain).
  - Sinusoid einsum as f32 PE matmuls with the WEIGHT CHUNK STATIONARY
    (lhsT = W[rows, 128 d-cols]) and the block-diagonal feature matrix moving
    (rhs [rows, 24]): the f32 4-cycles/row penalty applies to the 24-wide
    moving operand instead of a 512-wide one, and the result lands directly
    in d-on-partitions layout (no transpose). Accumulation order over the
    contraction rows/chunks is identical to the reference einsum.
  - cos(p+b)*sin(p) via ScalarE Sin with explicit range reduction in cycle
    units: m = u - rint(u); Sin(2*pi*m) = sin(2*pi*u); bsh carries b/(2*pi)
    + 0.25 so the cos becomes the same shifted sin.
  - combine + hard_quantize on DVE, transpose back, DMA out.

Perf shape: per-DMA overhead (HWDGE + issuing-sequencer hold) is ~0.7us, so
tables and weights are HOST-PACKED into one contiguous [partitions, bytes]
block per 512-wide chunk -> exactly 2 DMAs per chunk. DVE dispatch is
~0.4us/op, so the fold tree and the trig chain are batched across groups of
5 chunks. Host does only O(N*levels + K*D) layout prep: index math
(bit-identical to the reference's f32 ops), one-hot construction, weight
restack/padding, and sharding.
"""

import sys

for _p in ("/opt/trn_rl_repo",):
    if _p not in sys.path:
        sys.path.insert(0, _p)

import numpy as np

import concourse.bacc as bacc
import concourse.mybir as mybir
import concourse.tile as tile
from concourse import bass_utils, masks

F32 = mybir.dt.float32
I32 = mybir.dt.int32
BF = mybir.dt.bfloat16
FP8 = mybir.dt.float8e4
FP8NP = mybir.dt.np(FP8)
AF = mybir.AluOpType

D = 100000          # true hypervector dim
NCORES = 8
DC = 12800          # per-core padded dim
DP = DC * NCORES    # 102400
N = 128             # window length
LEVELS = 100
TIMESTAMPS = 128
CH = 25             # chunks of 512 per core
CW = 512            # chunk width
GRP = 5             # chunks per fold/trig batch group
NSUB = CH * 4       # 100 sub-chunks of 128
NK = 24             # sinusoid kernels (6 big + 18 small)
KROWS = 600         # stacked contraction dim (6*91 + 18*3)
KB = 5              # contraction row blocks
KR = KROWS // KB    # 120 rows per block

_TWO_PI = np.float32(2.0 * np.pi)
_INV_2PI = np.float32(1.0 / (2.0 * np.pi))

_nc_cache = {}

# (row0, nrows) blocks of the stacked table tensor: x, y, z, t
TBLOCKS = [(0, LEVELS), (LEVELS, LEVELS), (2 * LEVELS, LEVELS), (3 * LEVELS, TIMESTAMPS)]


def _build_nc():
    nc = bacc.Bacc("TRN2", target_bir_lowering=False, debug=False)

    # tabs4: per chunk c, cols [c*2048, (c+1)*2048) hold the 4 table blocks
    # (x,y,z,t), each [rows<=128 on partitions, 512], zero-padded to 128 rows.
    tabs4 = nc.dram_tensor("tabs4", [128, CH * 4 * CW], FP8, kind="ExternalInput")
    # oh4: 4 one-hot lhsT blocks [rows, 128] at col b*128
    oh4 = nc.dram_tensor("oh4", [128, 4 * N], FP8, kind="ExternalInput")
    # wpack: per chunk c, cols [c*2560, (c+1)*2560) hold 5 K-blocks of
    # [120 rows on partitions, 512 d]
    wpack = nc.dram_tensor("wpack", [KR, CH * KB * CW], F32, kind="ExternalInput")
    # fbd2: 5 K-blocks of the block-diagonal feature matrix, [120, 24] each
    fbd2 = nc.dram_tensor("fbd2", [KR, KB * NK], F32, kind="ExternalInput")
    bsh = nc.dram_tensor("bsh", [N, NSUB * NK], F32, kind="ExternalInput")
    out = nc.dram_tensor("out", [NSUB, N], F32, kind="ExternalOutput")

    with tile.TileContext(nc) as tc:
        with (
            tc.tile_pool(name="const", bufs=1) as constp,
            tc.tile_pool(name="grand", bufs=1) as grandp,
        ):
            ident_bf = constp.tile([128, 128], BF)
            masks.make_identity(nc, ident_bf[:])
            ident_f32 = constp.tile([128, 128], F32)
            masks.make_identity(nc, ident_f32[:])

            oh_sb = constp.tile([128, 4 * N], FP8, tag="oh4")
            nc.sync.dma_start(oh_sb[:], oh4.ap())
            fbd_sb = constp.tile([KR, KB * NK], F32, tag="fbd2")
            nc.sync.dma_start(fbd_sb[:], fbd2.ap())


            outq_all = grandp.tile([128, NSUB], F32, tag="outq_all")

            with (
                tc.tile_pool(name="tabs", bufs=4) as tabp,
                tc.tile_pool(name="wts", bufs=4) as wp,
                tc.tile_pool(name="binds", bufs=3) as bindp,
                tc.tile_pool(name="bt5s", bufs=2) as btp,
                tc.tile_pool(name="folds", bufs=2) as foldp,
                tc.tile_pool(name="trig", bufs=2) as trp,
                tc.tile_pool(name="comb", bufs=2) as cp,
                tc.tile_pool(name="psA", bufs=2, space="PSUM") as psa,
                tc.tile_pool(name="psB", bufs=2, space="PSUM") as psb,
            ):
                NG = CH // GRP
                GW = GRP * 4 * NK

                def emit_tail(grp, gs, ge, ppt_g, bt5, bsh_t, hv_t, trig_ts, comb_ts):
                    """Fold + trig + combine for chunks [gs, ge) of group grp.

                    Slice-wise identical to processing the whole group at
                    once: the fold tree is over the window axis within each
                    128-wide sub-block, and trig/combine are elementwise.
                    """
                    u, i1, m1, s1, u2, i2, m2, s2, fg_t = trig_ts
                    a1, q1, a2, a3, p2, q2, comb = comb_ts
                    nb = 4 * (ge - gs)
                    ts = slice(gs * 4 * NK, ge * 4 * NK)
                    qs = slice(gs * 4, ge * 4)

                    # fold tree over the window dim (free axis)
                    src = bt5[:, gs * CW : ge * CW].rearrange(
                        "p (s n) -> p s n", s=nb
                    )
                    w = 64
                    while w >= 1:
                        if w == 1:
                            dst_ap = hv_t[:, qs].rearrange("p (s n) -> p s n", n=1)
                        else:
                            t_new = foldp.tile(
                                [128, nb * w], F32,
                                tag=f"fold{nb}_{w}", name=f"fold{nb}_{w}",
                            )
                            dst_ap = t_new[:].rearrange("p (s n) -> p s n", s=nb)
                        nc.vector.tensor_mul(
                            dst_ap, src[:, :, 0:w], src[:, :, w : 2 * w]
                        )
                        if w > 1:
                            src = dst_ap
                        w //= 2

                    # trig: range reduction in cycle units: r = u - rint(u)
                    # in [-0.5, 0.5] (DVE f32->int32 copy rounds half-to-even,
                    # and the subtraction is exact), then Sin(2*pi*r) =
                    # sin(2*pi*u) on ScalarE's [-pi, pi] domain; the 2*pi
                    # multiply is fused into the activation's scale (same
                    # f32 product the reference rounds).
                    nc.vector.tensor_scalar_mul(u[:, ts], ppt_g[:, ts], float(_INV_2PI))
                    nc.vector.tensor_copy(i1[:, ts], u[:, ts])
                    nc.vector.tensor_sub(m1[:, ts], u[:, ts], i1[:, ts])
                    nc.scalar.activation(
                        s1[:, ts], m1[:, ts], mybir.ActivationFunctionType.Sin,
                        scale=float(_TWO_PI),
                    )
                    nc.vector.tensor_add(u2[:, ts], u[:, ts], bsh_t[:, ts])
                    nc.vector.tensor_copy(i2[:, ts], u2[:, ts])
                    nc.vector.tensor_sub(m2[:, ts], u2[:, ts], i2[:, ts])
                    nc.scalar.activation(
                        s2[:, ts], m2[:, ts], mybir.ActivationFunctionType.Sin,
                        scale=float(_TWO_PI),
                    )
                    nc.vector.tensor_mul(fg_t[:, ts], s2[:, ts], s1[:, ts])

                    # combine + hard quantize. t2's factor chains run on the
                    # (otherwise idle) Pool engine; t1's on DVE. All ops are
                    # scalar IEEE f32 mul/add, same order as the reference.
                    f3 = fg_t[:].rearrange("p (s k) -> p s k", k=NK)

                    def f(k):
                        return f3[:, qs, k : k + 1]

                    def v(t):
                        return t[:].rearrange("p (s k) -> p s k", k=1)[:, qs, :]

                    hvv = v(hv_t)
                    a1v = v(a1)
                    nc.vector.tensor_add(a1v, f(6), f(21))
                    nc.vector.tensor_add(a1v, a1v, f(23))
                    q1v = v(q1)
                    nc.vector.tensor_mul(q1v, hvv, a1v)
                    a2v = v(a2)
                    nc.vector.tensor_add(a2v, f(9), f(10))
                    nc.vector.tensor_mul(q1v, q1v, a2v)
                    for k in (11, 12, 17, 18):
                        nc.vector.tensor_mul(q1v, q1v, f(k))

                    a3v = v(a3)
                    nc.gpsimd.tensor_add(a3v, f(6), f(10))
                    nc.gpsimd.tensor_add(a3v, a3v, f(11))
                    nc.gpsimd.tensor_add(a3v, a3v, f(12))
                    p2v = v(p2)
                    nc.gpsimd.tensor_mul(p2v, f(0), f(1))
                    for k in (2, 3, 4, 5):
                        nc.gpsimd.tensor_mul(p2v, p2v, f(k))
                    q2v = v(q2)
                    nc.gpsimd.tensor_mul(q2v, hvv, a3v)
                    nc.gpsimd.tensor_mul(q2v, q2v, p2v)

                    nc.vector.tensor_add(v(comb), q1v, q2v)
                    cs2 = comb[:, qs]
                    nc.vector.tensor_scalar(cs2, cs2, 0.0, 2.0, AF.is_gt, AF.mult)
                    nc.vector.tensor_scalar(
                        outq_all[:, grp * 4 * GRP + gs * 4 : grp * 4 * GRP + ge * 4],
                        cs2, -1.0, None, AF.add,
                    )

                for grp in range(NG):
                    ppt_g = psb.tile([128, GRP * 4 * NK], F32, tag="ppt")
                    bt5 = btp.tile([128, GRP * CW], BF, tag="bt5")
                    bsh_t = trp.tile([N, GW], F32, tag="bsh_t")
                    nc.sync.dma_start(
                        bsh_t[:], bsh.ap()[:, grp * GW : (grp + 1) * GW]
                    )
                    hv_t = foldp.tile([128, 4 * GRP], F32, tag="hv")
                    u = trp.tile([128, GW], F32, tag="u")
                    i1 = trp.tile([128, GW], I32, tag="i1")
                    m1 = trp.tile([128, GW], F32, tag="m1")
                    s1 = trp.tile([128, GW], F32, tag="s1")
                    u2 = trp.tile([128, GW], F32, tag="u2")
                    i2 = trp.tile([128, GW], I32, tag="i2")
                    m2 = trp.tile([128, GW], F32, tag="m2")
                    s2 = trp.tile([128, GW], F32, tag="s2")
                    fg_t = trp.tile([128, GW], F32, tag="fg")
                    trig_ts = (u, i1, m1, s1, u2, i2, m2, s2, fg_t)
                    comb_ts = tuple(
                        cp.tile([128, 4 * GRP], F32, tag=t, name=t)
                        for t in ("a1", "q1", "a2", "a3", "p2", "q2", "comb")
                    )
                    for g in range(GRP):
                        c = grp * GRP + g
                        tab_t = tabp.tile([128, 4 * CW], FP8, tag="tab")
                        nc.sync.dma_start(
                            tab_t[:], tabs4.ap()[:, c * 4 * CW : (c + 1) * 4 * CW]
                        )
                        w_t = wp.tile([KR, KB * CW], F32, tag="w")
                        nc.scalar.dma_start(
                            w_t[:], wpack.ap()[:, c * KB * CW : (c + 1) * KB * CW]
                        )

                        # ---- phase A: lookups, bind ----------------------
                        ps3 = psa.tile([128, CW], F32, tag="ps3")
                        for b in range(3):
                            nc.tensor.matmul(
                                ps3[:],
                                oh_sb[0:LEVELS, b * N : (b + 1) * N],
                                tab_t[0:LEVELS, b * CW : (b + 1) * CW],
                                start=(b == 0),
                                stop=(b == 2),
                            )
                        ptl = psa.tile([128, CW], F32, tag="ptl")
                        nc.tensor.matmul(
                            ptl[:],
                            oh_sb[0:TIMESTAMPS, 3 * N : 4 * N],
                            tab_t[0:TIMESTAMPS, 3 * CW : 4 * CW],
                            start=True,
                            stop=True,
                        )

                        tl_sb = bindp.tile([128, CW], BF, tag="tl_sb")
                        nc.scalar.copy(tl_sb[:], ptl[:])
                        bind_sb = bindp.tile([128, CW], BF, tag="bind_sb")
                        nc.vector.tensor_mul(bind_sb[:], ps3[:], tl_sb[:])

                        pbt = psa.tile([128, CW], BF, tag="pbt")
                        for s in range(4):
                            ss = slice(s * 128, (s + 1) * 128)
                            nc.tensor.matmul(
                                pbt[:, ss], bind_sb[:, ss], ident_bf[:],
                                is_transpose=True,
                            )
                        nc.scalar.copy(bt5[:, g * CW : (g + 1) * CW], pbt[:])

                        # ---- phase B: einsum, W chunk stationary ---------
                        for s in range(4):
                            od = slice(g * 4 * NK + s * NK, g * 4 * NK + (s + 1) * NK)
                            for i in range(KB):
                                nc.tensor.matmul(
                                    ppt_g[:, od],
                                    w_t[:, i * CW + s * 128 : i * CW + (s + 1) * 128],
                                    fbd_sb[:, i * NK : (i + 1) * NK],
                                    start=(i == 0),
                                    stop=(i == KB - 1),
                                )

                    # ---- group tail: fold tree over the window dim -------
                    src = bt5[:].rearrange("p (s n) -> p s n", s=4 * GRP)
                    hv_t = foldp.tile([128, 4 * GRP], F32, tag="hv")
                    w = 64
                    while w >= 1:
                        if w == 1:
                            dst_ap = hv_t[:].rearrange("p (s n) -> p s n", n=1)
                        else:
                            t_new = foldp.tile([128, 4 * GRP * w], F32, tag=f"fold{w}")
                            dst_ap = t_new[:].rearrange("p (s n) -> p s n", s=4 * GRP)
                        nc.vector.tensor_mul(
                            dst_ap, src[:, :, 0:w], src[:, :, w : 2 * w]
                        )
                        if w > 1:
                            src = dst_ap
                        w //= 2

                    # ---- group tail: trig ---------------------------------
                    # range reduction in cycle units: r = u - rint(u) in
                    # [-0.5, 0.5] (DVE f32->int32 copy rounds half-to-even,
                    # and the subtraction is exact), then Sin(2*pi*r) =
                    # sin(2*pi*u) on ScalarE's [-pi, pi] domain; the 2*pi
                    # multiply is fused into the activation's scale (same
                    # f32 product the reference rounds).
                    u = trp.tile([128, GW], F32, tag="u")
                    nc.vector.tensor_scalar_mul(u[:], ppt_g[:], float(_INV_2PI))
                    i1 = trp.tile([128, GW], I32, tag="i1")
                    nc.vector.tensor_copy(i1[:], u[:])
                    m1 = trp.tile([128, GW], F32, tag="m1")
                    nc.vector.tensor_sub(m1[:], u[:], i1[:])
                    s1 = trp.tile([128, GW], F32, tag="s1")
                    nc.scalar.activation(
                        s1[:], m1[:], mybir.ActivationFunctionType.Sin,
                        scale=float(_TWO_PI),
                    )
                    u2 = trp.tile([128, GW], F32, tag="u2")
                    nc.vector.tensor_add(u2[:], u[:], bsh_t[:])
                    i2 = trp.tile([128, GW], I32, tag="i2")
                    nc.vector.tensor_copy(i2[:], u2[:])
                    m2 = trp.tile([128, GW], F32, tag="m2")
                    nc.vector.tensor_sub(m2[:], u2[:], i2[:])
                    s2 = trp.tile([128, GW], F32, tag="s2")
                    nc.scalar.activation(
                        s2[:], m2[:], mybir.ActivationFunctionType.Sin,
                        scale=float(_TWO_PI),
                    )
                    fg_t = trp.tile([128, GW], F32, tag="fg")
                    nc.vector.tensor_mul(fg_t[:], s2[:], s1[:])

                    # ---- group tail: combine + hard quantize --------------
                    # t2's factor chains run on the (otherwise idle) Pool
                    # engine; t1's on DVE. All ops are scalar IEEE f32
                    # mul/add, same order as the reference formula.
                    f3 = fg_t[:].rearrange("p (s k) -> p s k", k=NK)

                    def f(k):
                        return f3[:, :, k : k + 1]

                    def tmp(tag):
                        return cp.tile([128, 4 * GRP], F32, tag=tag, name=tag)

                    hvv = hv_t[:].rearrange("p (s k) -> p s k", k=1)
                    a1 = tmp("a1")
                    a1v = a1[:].rearrange("p (s k) -> p s k", k=1)
                    nc.vector.tensor_add(a1v, f(6), f(21))
                    nc.vector.tensor_add(a1v, a1v, f(23))
                    q1 = tmp("q1")
                    q1v = q1[:].rearrange("p (s k) -> p s k", k=1)
                    nc.vector.tensor_mul(q1v, hvv, a1v)
                    a2 = tmp("a2")
                    a2v = a2[:].rearrange("p (s k) -> p s k", k=1)
                    nc.vector.tensor_add(a2v, f(9), f(10))
                    nc.vector.tensor_mul(q1v, q1v, a2v)
                    for k in (11, 12, 17, 18):
                        nc.vector.tensor_mul(q1v, q1v, f(k))

                    a3 = tmp("a3")
                    a3v = a3[:].rearrange("p (s k) -> p s k", k=1)
                    nc.gpsimd.tensor_add(a3v, f(6), f(10))
                    nc.gpsimd.tensor_add(a3v, a3v, f(11))
                    nc.gpsimd.tensor_add(a3v, a3v, f(12))
                    p2 = tmp("p2")
                    p2v = p2[:].rearrange("p (s k) -> p s k", k=1)
                    nc.gpsimd.tensor_mul(p2v, f(0), f(1))
                    for k in (2, 3, 4, 5):
                        nc.gpsimd.tensor_mul(p2v, p2v, f(k))
                    q2 = tmp("q2")
                    q2v = q2[:].rearrange("p (s k) -> p s k", k=1)
                    nc.gpsimd.tensor_mul(q2v, hvv, a3v)
                    nc.gpsimd.tensor_mul(q2v, q2v, p2v)

                    comb = tmp("comb")
                    nc.vector.tensor_add(comb[:], q1[:], q2[:])
                    oq = slice(grp * 4 * GRP, (grp + 1) * 4 * GRP)
                    nc.vector.tensor_scalar(
                        comb[:], comb[:], 0.0, 2.0, AF.is_gt, AF.mult
                    )
                    nc.vector.tensor_scalar(
                        outq_all[:, oq], comb[:], -1.0, None, AF.add
                    )

            # ---------------- transpose back + output -------------------------
            with (
                tc.tile_pool(name="outp", bufs=1) as cpo,
                tc.tile_pool(name="psC", bufs=1, space="PSUM") as psc,
            ):
                pso = psc.tile([NSUB, 128], F32, tag="pso")
                nc.tensor.matmul(
                    pso[:], outq_all[:], ident_f32[:], is_transpose=True
                )
                out_sb = cpo.tile([NSUB, 128], F32, tag="out_sb")
                nc.scalar.copy(out_sb[:], pso[:])
                nc.sync.dma_start(out.ap(), out_sb[:])

    nc.compile()
    return nc


def _get_nc():
    if "nc" not in _nc_cache:
        _nc_cache["nc"] = _build_nc()
    return _nc_cache["nc"]


def _value_to_index(x, low, high, num):
    """Bit-identical (f32 elementwise IEEE ops) to the reference's jnp math."""
    x = x.astype(np.float32)
    xc = np.clip(x, np.float32(low), np.float32(high))
    t = (xc - np.float32(low)) / np.float32(high - low) * np.float32(num - 1)
    idx = np.round(t)  # round-half-even, same as jnp.round
    return np.clip(idx, 0, num - 1).astype(np.int32)


def prepare_in_maps(
    input,
    feat,
    level_x,
    level_y,
    level_z,
    level_t,
    W_big,
    b_big,
    W_small,
    b_small,
):
    ix = _value_to_index(input[:, 1], -5.0, 5.0, LEVELS)
    iy = _value_to_index(input[:, 2], -5.0, 5.0, LEVELS)
    iz = _value_to_index(input[:, 3], -5.0, 5.0, LEVELS)
    it = _value_to_index(input[:, 0], 0.0, float(TIMESTAMPS), TIMESTAMPS)

    # one-hot lhsT blocks [rows, 128] packed at col b*128 (0/1 exact in fp8)
    oh4 = np.zeros((128, 4 * N), dtype=FP8NP)
    for bi, idx in enumerate((ix, iy, iz, it)):
        oh4[idx, bi * N + np.arange(N)] = 1

    featb = feat[:546].reshape(6, 91).astype(np.float32)
    feats = feat[546:600].reshape(18, 3).astype(np.float32)
    fbd = np.zeros((KROWS, NK), dtype=np.float32)
    for k in range(6):
        fbd[k * 91 : (k + 1) * 91, k] = featb[k]
    for k in range(18):
        fbd[546 + k * 3 : 546 + (k + 1) * 3, 6 + k] = feats[k]
    # 5 row-blocks of 120, side by side: [120, 5*24]
    fbd2 = np.ascontiguousarray(
        fbd.reshape(KB, KR, NK).transpose(1, 0, 2).reshape(KR, KB * NK)
    )

    def padD(a):
        w = [(0, 0)] * a.ndim
        w[-1] = (0, DP - D)
        return np.pad(a, w)

    tables = [
        padD(t).astype(FP8NP) for t in (level_x, level_y, level_z, level_t)
    ]

    # W stack [600, DP] f32: rows = (kernel-major, in-feature) of W_big/W_small
    wb = np.ascontiguousarray(W_big.transpose(0, 2, 1)).reshape(546, D)
    ws = np.ascontiguousarray(W_small.transpose(0, 2, 1)).reshape(54, D)
    wstk = padD(np.concatenate([wb, ws], axis=0)).astype(np.float32)

    # b shift in cycles (+0.25 for the cos->sin shift), d-on-partitions layout
    ball = np.concatenate([b_big, b_small], axis=0).astype(np.float64)
    bsh_full = padD((ball / (2.0 * np.pi) + 0.25).astype(np.float32))  # [24, DP]

    in_maps = []
    for ci in range(NCORES):
        ds = slice(ci * DC, (ci + 1) * DC)
        # pack the 4 table blocks: [128, CH, 4, 512]
        tabs4 = np.zeros((128, CH, 4, CW), dtype=FP8NP)
        for bi, tab in enumerate(tables):
            rn = TBLOCKS[bi][1]
            tabs4[0:rn, :, bi, :] = tab[:, ds].reshape(rn, CH, CW)
        # pack the 5 W row-blocks: [120, CH, 5, 512]
        wp_ = np.empty((KR, CH, KB, CW), dtype=np.float32)
        wc = wstk[:, ds]
        for i in range(KB):
            wp_[:, :, i, :] = wc[i * KR : (i + 1) * KR].reshape(KR, CH, CW)
        bs = (
            bsh_full[:, ds]
            .reshape(NK, NSUB, 128)
            .transpose(2, 1, 0)
            .reshape(128, NSUB * NK)
        )
        in_maps.append(
            {
                "tabs4": np.ascontiguousarray(tabs4.reshape(128, CH * 4 * CW)),
                "oh4": oh4,
                "wpack": np.ascontiguousarray(wp_.reshape(KR, CH * KB * CW)),
                "fbd2": fbd2,
                "bsh": np.ascontiguousarray(bs),
            }
        )
    return in_maps


def kernel(**inputs):
    nc = _get_nc()
    inputs = {k: np.asarray(v) for k, v in inputs.items()}
    in_maps = prepare_in_maps(**inputs)
    _nc_cache["last_in_maps"] = in_maps
    res = bass_utils.run_bass_kernel_spmd(nc, in_maps, core_ids=list(range(NCORES)))
    shards = [res.results[ci]["out"].reshape(-1) for ci in range(NCORES)]
    return np.concatenate(shards)[:D].astype(np.float32)
